# revision 19
# baseline (speedup 1.0000x reference)
"""Trainium2 Bass kernel for BatteryMoEFlattenIntraCycleMoELayer.

Math: out[b,l,d] = sum_e gates[b,e] * (x[b,l,:] @ W[e] + bias[e])
with gates = renormalized masked softmax(logits), x = curves flattened to
[B, L, 900]; plus a scalar guide_loss computed from logits/masks only.

Strategy:
  * Data-parallel over B across 8 NeuronCores (16 rows each, 1600 (b,l)
    columns per core).
  * The expert combine is folded INTO the matmul contraction: per core,
    out^T[d, col] = sum_{e,i} W[e][i,d] * (xT[i,col] * g[col,e]) + bias
    i.e. one accumulation group over K = 8*900 (+8 bias rows) per PSUM tile.
    The gate-scaled moving operand is built on-chip by VectorE (bf16 2x mode)
    right before TensorE consumes it; bias rides as a final K=8 chunk whose
    moving operand is the raw gate rows. No separate combine pass, no
    collectives.
  * bf16 matmul inputs (fp32 matmul costs 4x on PE), fp32 PSUM accumulation,
    bf16 output. Host does the tiny [128,8] softmax/guide_loss and the x
    transpose/pack.

Layout per core:
  xt   [896,1600]  x^T rows 0..895 (features on partitions)
  xtl  [32,1600]   x^T rows 896..899 replicated per expert (tail merge)
  wm   [56,128,512] W[e][128j:128j+128,:] chunks, e-major
  wtl  [32,512]    W[e][896+r,:] tail rows, matching xtl row order
  gbc  [8,128,1600] gate row of expert e replicated across 128 partitions
  gtl  [32,1600]   gate rows matching xtl
  gt   [8,1600]    raw gate rows (bias chunk moving operand)
  bt   [8,512]     bias (bias chunk stationary operand)
  out_t [512,1600] out^T
"""

import sys
from contextlib import ExitStack

import numpy as np

if "/opt/trn_rl_repo" not in sys.path:
    sys.path.insert(0, "/opt/trn_rl_repo")

import ml_dtypes

import concourse.bacc as bacc
import concourse.bass as bass
import concourse.mybir as mybir
from concourse.bass_utils import run_bass_kernel_spmd
from concourse.tile import TileContext

BF = ml_dtypes.bfloat16

B, L, CL, E, D = 128, 100, 300, 8, 512
FAN = 3 * CL                 # 900
NCORES = 8
BLOC = B // NCORES           # 16
NL = BLOC * L                # 1600 moving columns per core
JF = FAN // 128              # 7 full 128-row chunks per expert
TK = FAN - JF * 128          # 4 tail rows per expert
TROWS = E * TK               # 32 merged tail rows
# the small wave runs FIRST: it carries all the first-use DMA dependencies
# with per-chunk-unique rhs slots (so no PE WAR wait can stack on top of the
# DMA wait — walrus TT instructions allow a single sync-wait)
WAVES = [(1536, 64), (0, 512), (512, 512), (1024, 512)]

_PROG = None
LAST_RESULT = None  # BassKernelResults of the most recent run (for test harness)
LAST_IN_MAPS = None  # per-core input maps of the most recent run


def _build_program():
    # Bacc (not raw Bass): its finalize() runs generate_event_semaphores,
    # which legalizes >1-sync-wait instructions for TRN2 hardware.
    nc = bacc.Bacc()
    bf = mybir.dt.bfloat16
    xt = nc.declare_dram_parameter("xt", [JF * 128, NL], bf, isOutput=False)
    # tail rows of x^T and their gate rows packed side-by-side so the tail
    # tensor_mul depends on a single DMA (walrus caps TT sync-waits at 3)
    xgtl = nc.declare_dram_parameter("xgtl", [TROWS, 2 * NL], bf, isOutput=False)
    wm = nc.declare_dram_parameter("wm", [E * JF, 128, D], bf, isOutput=False)
    wtl = nc.declare_dram_parameter("wtl", [TROWS, D], bf, isOutput=False)
    bt = nc.declare_dram_parameter("bt", [E, D], bf, isOutput=False)
    gbc = nc.declare_dram_parameter("gbc", [E, 128, NL], bf, isOutput=False)
    gt = nc.declare_dram_parameter("gt", [E, NL], bf, isOutput=False)
    out_t = nc.declare_dram_parameter("out_t", [D, NL], bf, isOutput=True)

    with TileContext(nc) as tc, ExitStack() as ctx:
        const = ctx.enter_context(tc.tile_pool(name="const", bufs=1))
        rhsp = ctx.enter_context(tc.tile_pool(name="rhsp", bufs=6))
        psp = ctx.enter_context(tc.tile_pool(name="psp", bufs=2, space="PSUM"))
        # 16 bufs = one slot per evacuation: no slot reuse, so ACT copies
        # never pick up a third (DMA-out WAR) sync-wait — walrus caps waits
        outp = ctx.enter_context(tc.tile_pool(name="outp", bufs=16))

        # ---- resident loads, in rough consumption order ----
        xt_sb = []
        for j in range(JF):
            t = const.tile([128, NL], bf, tag=f"xt{j}", name=f"xt{j}")
            nc.sync.dma_start(out=t[:], in_=xt[j * 128:(j + 1) * 128, :])
            xt_sb.append(t)
        xgtl_sb = const.tile([TROWS, 2 * NL], bf, tag="xgtl", name="xgtl_sb")
        nc.sync.dma_start(out=xgtl_sb[:], in_=xgtl[:])
        gt_sb = const.tile([E, NL], bf, tag="gt", name="gt_sb")
        nc.sync.dma_start(out=gt_sb[:], in_=gt[:])
        bt_sb = const.tile([E, D], bf, tag="bt", name="bt_sb")
        nc.sync.dma_start(out=bt_sb[:], in_=bt[:])

        gbc_sb, w_sb = [], []
        for e in range(E):
            g = const.tile([128, NL], bf, tag=f"g{e}", name=f"g{e}")
            nc.sync.dma_start(out=g[:], in_=gbc[e])
            gbc_sb.append(g)
            for j in range(JF):
                w = const.tile([128, D], bf, tag=f"w{e}_{j}", name=f"w{e}_{j}")
                nc.sync.dma_start(out=w[:], in_=wm[e * JF + j])
                w_sb.append(w)
        wtl_sb = const.tile([TROWS, D], bf, tag="wtl", name="wtl_sb")
        nc.sync.dma_start(out=wtl_sb[:], in_=wtl[:])

        # (stationary, moving-src, gate-src, K, gate-col-offset)
        chunks = [(w_sb[e * JF + j], xt_sb[j], gbc_sb[e], 128, 0)
                  for e in range(E) for j in range(JF)]
        chunks.append((wtl_sb, xgtl_sb, xgtl_sb, TROWS, NL))

        # One-element DVE "pre-touch" per DMA-loaded tile feeding a
        # tensor_mul, emitted at first use: it absorbs the DMA-queue wait
        # into the DVE vector clock so real muls only wait on {PE, DVE}
        # (walrus TT instructions support at most 2 sync waits).
        touched = {}

        def touch(tile_ap):
            key = id(tile_ap)
            if key not in touched:
                touched[key] = tile_ap  # keep ref so id() stays unique
                n = len(touched)
                tt = const.tile([1, 16], bf, tag=f"touch{n}", name=f"touch{n}")
                nc.vector.tensor_copy(tt[0:1, 0:1], tile_ap[0:1, 0:1])

        touch(gt_sb)
        touch(bt_sb)
        for wi, (c0, cw) in enumerate(WAVES):
            psums = [psp.tile([128, cw], mybir.dt.float32, tag=f"ps{mi}",
                              name=f"ps{mi}_{c0}") for mi in range(4)]
            for ci, (wt, xs, gs, kk, goff) in enumerate(chunks):
                touch(xs)
                touch(gs)
                if wi == 0:
                    # unique slot per chunk in the first (tiny) wave
                    rhs = rhsp.tile([128, cw], bf, tag=f"rhs0_{ci}",
                                    name=f"rhs{c0}_{ci}", bufs=1)
                else:
                    rhs = rhsp.tile([128, cw], bf, tag="rhs",
                                    name=f"rhs{c0}_{ci}", bufs=6)
                nc.vector.tensor_mul(rhs[:kk, :], xs[:kk, c0:c0 + cw],
                                     gs[:kk, goff + c0:goff + c0 + cw])
                for mi in range(4):
                    nc.tensor.matmul(psums[mi][:, :],
                                     lhsT=wt[:kk, mi * 128:(mi + 1) * 128],
                                     rhs=rhs[:kk, :],
                                     start=(ci == 0), stop=False)
            for mi in range(4):
                nc.tensor.matmul(psums[mi][:, :],
                                 lhsT=bt_sb[:, mi * 128:(mi + 1) * 128],
                                 rhs=gt_sb[:, c0:c0 + cw],
                                 start=False, stop=True)
            for mi in range(4):
                ot = outp.tile([128, cw], bf, tag="ot", name=f"ot{c0}_{mi}")
                nc.scalar.copy(ot[:], psums[mi][:, :])
                # chain the ACT evacuation into the DVE clock so later-wave
                # matmuls reusing this PSUM slot carry a single DVE wait
                touch(ot)
                nc.sync.dma_start(out=out_t[mi * 128:(mi + 1) * 128, c0:c0 + cw],
                                  in_=ot[:])
    nc.finalize()  # Bacc: runs compile() passes incl. sync-wait legalization
    return nc


def _gates_and_loss(logits, moe_masks):
    logits = np.asarray(logits, np.float32)
    mask = (np.asarray(moe_masks) == 1).astype(np.float32)
    ex = np.exp(logits - logits.max(axis=1, keepdims=True))
    raw = ex / ex.sum(axis=1, keepdims=True)
    gated = raw * mask
    gates = gated / (gated.sum(axis=1, keepdims=True) + np.float32(1e-9))
    sum_masked_raw = np.float32(np.sum(raw * mask) / np.float32(B))
    guide_loss = np.float32((np.float32(1.0) - sum_masked_raw) ** 2)
    return gates, guide_loss


def kernel(cycle_curve_data, logits, moe_masks, W, b):
    global _PROG, LAST_RESULT
    if _PROG is None:
        _PROG = _build_program()

    x = np.asarray(cycle_curve_data, np.float32).reshape(B, L, FAN)
    gates, guide_loss = _gates_and_loss(logits, moe_masks)

    Wb = np.asarray(W, np.float32).astype(BF)          # [E, FAN, D]
    wm = np.ascontiguousarray(Wb[:, :JF * 128, :]).reshape(E * JF, 128, D)
    wtl = np.ascontiguousarray(Wb[:, JF * 128:, :]).reshape(TROWS, D)
    bt = np.asarray(b, np.float32).astype(BF)

    in_maps = []
    for c in range(NCORES):
        xc = x[c * BLOC:(c + 1) * BLOC].reshape(NL, FAN)
        xT = np.ascontiguousarray(xc.T).astype(BF)      # [900, 1600]
        grow = np.repeat(gates[c * BLOC:(c + 1) * BLOC], L, axis=0).T  # [E,NL] f32
        growb = grow.astype(BF)
        xgtl = np.concatenate(
            [np.tile(xT[JF * 128:], (E, 1)), np.repeat(growb, TK, axis=0)],
            axis=1)                                      # [32, 2*NL]
        in_maps.append({
            "xt": xT[:JF * 128],
            "xgtl": xgtl,
            "wm": wm,
            "wtl": wtl,
            "bt": bt,
            "gbc": np.ascontiguousarray(
                np.broadcast_to(growb[:, None, :], (E, 128, NL))),
            "gt": growb,
        })

    globals()["LAST_IN_MAPS"] = in_maps
    LAST_RESULT = run_bass_kernel_spmd(_PROG, in_maps, list(range(NCORES)))
    outs = []
    for c in range(NCORES):
        ot = LAST_RESULT.results[c]["out_t"]            # [512, 1600] bf16
        outs.append(np.ascontiguousarray(ot.T).reshape(BLOC, L, D))
    out = np.concatenate(outs, axis=0)                  # [128, 100, 512] bf16
    return out, guide_loss


# revision 21
# speedup vs baseline: 3.5261x; 3.5261x over previous
"""Trainium2 Bass kernel for BatteryMoEFlattenIntraCycleMoELayer.

Math: out[b,l,d] = sum_e gates[b,e] * (x[b,l,:] @ W[e] + bias[e])
with gates = renormalized masked softmax(logits), x = curves flattened to
[B, L, 900]; plus a scalar guide_loss computed from logits/masks only.

Strategy:
  * Data-parallel over B across 8 NeuronCores (16 rows each, 1600 (b,l)
    columns per core).
  * The expert combine is folded INTO the matmul contraction: per core,
    out^T[d, col] = sum_{e,i} W[e][i,d] * (xT[i,col] * g[col,e]) + bias
    i.e. one accumulation group over K = 8*900 (+8 bias rows) per PSUM tile.
    The gate-scaled moving operand is built on-chip by VectorE (bf16 2x mode)
    right before TensorE consumes it; bias rides as a final K=8 chunk whose
    moving operand is the raw gate rows. No separate combine pass, no
    collectives.
  * bf16 matmul inputs (fp32 matmul costs 4x on PE), fp32 PSUM accumulation,
    bf16 output. Host does the tiny [128,8] softmax/guide_loss and the x
    transpose/pack.

Layout per core:
  xt   [896,1600]  x^T rows 0..895 (features on partitions)
  xtl  [32,1600]   x^T rows 896..899 replicated per expert (tail merge)
  wm   [56,128,512] W[e][128j:128j+128,:] chunks, e-major
  wtl  [32,512]    W[e][896+r,:] tail rows, matching xtl row order
  gbc  [8,128,1600] gate row of expert e replicated across 128 partitions
  gtl  [32,1600]   gate rows matching xtl
  gt   [8,1600]    raw gate rows (bias chunk moving operand)
  bt   [8,512]     bias (bias chunk stationary operand)
  out_t [512,1600] out^T
"""

import sys
from contextlib import ExitStack

import numpy as np

if "/opt/trn_rl_repo" not in sys.path:
    sys.path.insert(0, "/opt/trn_rl_repo")

import ml_dtypes

import concourse.bacc as bacc
import concourse.bass as bass
import concourse.mybir as mybir
from concourse.bass_utils import run_bass_kernel_spmd
from concourse.tile import TileContext

BF = ml_dtypes.bfloat16

B, L, CL, E, D = 128, 100, 300, 8, 512
FAN = 3 * CL                 # 900
NCORES = 8
BLOC = B // NCORES           # 16
NL = BLOC * L                # 1600 moving columns per core
JF = FAN // 128              # 7 full 128-row chunks per expert
TK = FAN - JF * 128          # 4 tail rows per expert
TROWS = E * TK               # 32 merged tail rows
# the small wave runs FIRST: it carries all the first-use DMA dependencies
# with per-chunk-unique rhs slots (so no PE WAR wait can stack on top of the
# DMA wait — walrus TT instructions allow a single sync-wait)
WAVES = [(1536, 64), (0, 512), (512, 512), (1024, 512)]

_PROG = None
LAST_RESULT = None  # BassKernelResults of the most recent run (for test harness)
LAST_IN_MAPS = None  # per-core input maps of the most recent run


def _build_program(repeat=1):
    # Bacc (not raw Bass): its finalize() runs generate_event_semaphores,
    # which legalizes >1-sync-wait instructions for TRN2 hardware.
    # repeat>1 duplicates the compute waves back-to-back inside one NEFF —
    # used by the test harness to measure steady-state compute time.
    nc = bacc.Bacc()
    bf = mybir.dt.bfloat16
    xt = nc.declare_dram_parameter("xt", [JF * 128, NL], bf, isOutput=False)
    # tail rows of x^T and their gate rows packed side-by-side so the tail
    # tensor_mul depends on a single DMA (walrus caps TT sync-waits at 3)
    xgtl = nc.declare_dram_parameter("xgtl", [TROWS, 2 * NL], bf, isOutput=False)
    wm = nc.declare_dram_parameter("wm", [E * JF, 128, D], bf, isOutput=False)
    wtl = nc.declare_dram_parameter("wtl", [TROWS, D], bf, isOutput=False)
    bt = nc.declare_dram_parameter("bt", [E, D], bf, isOutput=False)
    gbc = nc.declare_dram_parameter("gbc", [E, 128, NL], bf, isOutput=False)
    gt = nc.declare_dram_parameter("gt", [E, NL], bf, isOutput=False)
    out_t = nc.declare_dram_parameter("out_t", [D, NL], bf, isOutput=True)

    with TileContext(nc) as tc, ExitStack() as ctx:
        const = ctx.enter_context(tc.tile_pool(name="const", bufs=1))
        rhsp = ctx.enter_context(tc.tile_pool(name="rhsp", bufs=6))
        psp = ctx.enter_context(tc.tile_pool(name="psp", bufs=2, space="PSUM"))
        # 16 bufs = one slot per evacuation: no slot reuse, so ACT copies
        # never pick up a third (DMA-out WAR) sync-wait — walrus caps waits
        outp = ctx.enter_context(tc.tile_pool(name="outp", bufs=16))

        # ---- resident loads, in rough consumption order ----
        xt_sb = []
        for j in range(JF):
            t = const.tile([128, NL], bf, tag=f"xt{j}", name=f"xt{j}")
            nc.sync.dma_start(out=t[:], in_=xt[j * 128:(j + 1) * 128, :])
            xt_sb.append(t)
        xgtl_sb = const.tile([TROWS, 2 * NL], bf, tag="xgtl", name="xgtl_sb")
        nc.sync.dma_start(out=xgtl_sb[:], in_=xgtl[:])
        gt_sb = const.tile([E, NL], bf, tag="gt", name="gt_sb")
        nc.sync.dma_start(out=gt_sb[:], in_=gt[:])
        bt_sb = const.tile([E, D], bf, tag="bt", name="bt_sb")
        nc.sync.dma_start(out=bt_sb[:], in_=bt[:])

        gbc_sb, w_sb = [], []
        for e in range(E):
            g = const.tile([128, NL], bf, tag=f"g{e}", name=f"g{e}")
            nc.sync.dma_start(out=g[:], in_=gbc[e])
            gbc_sb.append(g)
            for j in range(JF):
                w = const.tile([128, D], bf, tag=f"w{e}_{j}", name=f"w{e}_{j}")
                nc.sync.dma_start(out=w[:], in_=wm[e * JF + j])
                w_sb.append(w)
        wtl_sb = const.tile([TROWS, D], bf, tag="wtl", name="wtl_sb")
        nc.sync.dma_start(out=wtl_sb[:], in_=wtl[:])

        # (stationary, moving-src, gate-src, K, gate-col-offset)
        chunks = [(w_sb[e * JF + j], xt_sb[j], gbc_sb[e], 128, 0)
                  for e in range(E) for j in range(JF)]
        chunks.append((wtl_sb, xgtl_sb, xgtl_sb, TROWS, NL))

        # One-element DVE "pre-touch" per DMA-loaded tile feeding a
        # tensor_mul, emitted at first use: it absorbs the DMA-queue wait
        # into the DVE vector clock so real muls only wait on {PE, DVE}
        # (walrus TT instructions support at most 2 sync waits).
        touched = {}

        def touch(tile_ap):
            key = id(tile_ap)
            if key not in touched:
                touched[key] = tile_ap  # keep ref so id() stays unique
                n = len(touched)
                tt = const.tile([1, 16], bf, tag=f"touch{n}", name=f"touch{n}")
                nc.vector.tensor_copy(tt[0:1, 0:1], tile_ap[0:1, 0:1])

        touch(gt_sb)
        touch(bt_sb)
        for wi, (c0, cw) in enumerate(WAVES * repeat):
            psums = [psp.tile([128, cw], mybir.dt.float32, tag=f"ps{mi}",
                              name=f"ps{mi}_{c0}") for mi in range(4)]
            for ci, (wt, xs, gs, kk, goff) in enumerate(chunks):
                touch(xs)
                touch(gs)
                if wi == 0:
                    # unique slot per chunk in the first (tiny) wave
                    rhs = rhsp.tile([128, cw], bf, tag=f"rhs0_{ci}",
                                    name=f"rhs{c0}_{ci}", bufs=1)
                else:
                    rhs = rhsp.tile([128, cw], bf, tag="rhs",
                                    name=f"rhs{c0}_{ci}", bufs=6)
                nc.vector.tensor_mul(rhs[:kk, :], xs[:kk, c0:c0 + cw],
                                     gs[:kk, goff + c0:goff + c0 + cw])
                for mi in range(4):
                    nc.tensor.matmul(psums[mi][:, :],
                                     lhsT=wt[:kk, mi * 128:(mi + 1) * 128],
                                     rhs=rhs[:kk, :],
                                     start=(ci == 0), stop=False)
            for mi in range(4):
                nc.tensor.matmul(psums[mi][:, :],
                                 lhsT=bt_sb[:, mi * 128:(mi + 1) * 128],
                                 rhs=gt_sb[:, c0:c0 + cw],
                                 start=False, stop=True)
            for mi in range(4):
                ot = outp.tile([128, cw], bf, tag="ot", name=f"ot{c0}_{mi}")
                nc.scalar.copy(ot[:], psums[mi][:, :])
                # chain the ACT evacuation into the DVE clock so later-wave
                # matmuls reusing this PSUM slot carry a single DVE wait
                touch(ot)
                nc.sync.dma_start(out=out_t[mi * 128:(mi + 1) * 128, c0:c0 + cw],
                                  in_=ot[:])
    nc.finalize()  # Bacc: runs compile() passes incl. sync-wait legalization
    return nc


def _gates_and_loss(logits, moe_masks):
    logits = np.asarray(logits, np.float32)
    mask = (np.asarray(moe_masks) == 1).astype(np.float32)
    ex = np.exp(logits - logits.max(axis=1, keepdims=True))
    raw = ex / ex.sum(axis=1, keepdims=True)
    gated = raw * mask
    gates = gated / (gated.sum(axis=1, keepdims=True) + np.float32(1e-9))
    sum_masked_raw = np.float32(np.sum(raw * mask) / np.float32(B))
    guide_loss = np.float32((np.float32(1.0) - sum_masked_raw) ** 2)
    return gates, guide_loss


def kernel(cycle_curve_data, logits, moe_masks, W, b):
    global _PROG, LAST_RESULT
    if _PROG is None:
        _PROG = _build_program()

    x = np.asarray(cycle_curve_data, np.float32).reshape(B, L, FAN)
    gates, guide_loss = _gates_and_loss(logits, moe_masks)

    Wb = np.asarray(W, np.float32).astype(BF)          # [E, FAN, D]
    wm = np.ascontiguousarray(Wb[:, :JF * 128, :]).reshape(E * JF, 128, D)
    wtl = np.ascontiguousarray(Wb[:, JF * 128:, :]).reshape(TROWS, D)
    bt = np.asarray(b, np.float32).astype(BF)

    in_maps = []
    for c in range(NCORES):
        xc = x[c * BLOC:(c + 1) * BLOC].reshape(NL, FAN)
        xT = np.ascontiguousarray(xc.T).astype(BF)      # [900, 1600]
        grow = np.repeat(gates[c * BLOC:(c + 1) * BLOC], L, axis=0).T  # [E,NL] f32
        growb = grow.astype(BF)
        xgtl = np.concatenate(
            [np.tile(xT[JF * 128:], (E, 1)), np.repeat(growb, TK, axis=0)],
            axis=1)                                      # [32, 2*NL]
        in_maps.append({
            "xt": xT[:JF * 128],
            "xgtl": xgtl,
            "wm": wm,
            "wtl": wtl,
            "bt": bt,
            "gbc": np.ascontiguousarray(
                np.broadcast_to(growb[:, None, :], (E, 128, NL))),
            "gt": growb,
        })

    globals()["LAST_IN_MAPS"] = in_maps
    LAST_RESULT = run_bass_kernel_spmd(_PROG, in_maps, list(range(NCORES)))
    outs = []
    for c in range(NCORES):
        ot = LAST_RESULT.results[c]["out_t"]            # [512, 1600] bf16
        outs.append(np.ascontiguousarray(ot.T).reshape(BLOC, L, D))
    out = np.concatenate(outs, axis=0)                  # [128, 100, 512] bf16
    return out, guide_loss
